# revision 18
# baseline (speedup 1.0000x reference)
"""Trainium2 Bass kernel for a Chemprop GNN message-passing layer.

Reference computation (single layer, n_nodes=50000, n_edges=300000, hidden=256):
    H   = relu(E)                                  # [E, 256]
    M_v = segment_sum(H, dest, n_nodes)            # [V, 256]
    out = (M_v[src] - H[rev]) @ W.T + b            # [E, 256]

Distribution over 8 NeuronCores (zero collectives):
  * Nodes are sharded: core c owns node range [c*6250, (c+1)*6250), padded to
    49 blocks of 128 lanes.
  * Phase 1 (segment sum): edges are grouped by dest-node ownership on the
    host; each core receives its edges' E-rows PRE-PERMUTED into
    (block, chunk, lane) slot order (a pure permutation + zero padding of E,
    i.e. a "dest-sorted edge sharding"). The device streams them
    contiguously, applies relu, and accumulates per 128-node block with
    one-hot selection matmuls: M_v_block += S_chunk.T @ H_chunk where
    S[e, n] = (dest_lane[e] == n), built on-device via is_equal vs an iota
    row. M_v lives entirely in SBUF (49 blocks x [128, 256]).
  * Phase 2 (gather-subtract-linear): edges are grouped by src-node
    ownership, so M_v[src] expansion is a local one-hot matmul
    Pv = R.T @ Mv_block with R[n, e] = (src_lane[e] == n). Only the
    reverse-edge term needs indirect gathers: E[rev] rows are fetched
    128-rows-per-instruction from a full replica of E in each core's DRAM.
    M_uv = Pv - relu(E[rev]) is transposed on the PE (two 128x128
    transposes) and multiplied by W.T via two accumulating matmuls; bias is
    fused into the PSUM->SBUF copy. Output rows are written contiguously in
    slot order; the host scatters them back to original edge order.
"""

import sys
from contextlib import ExitStack

import numpy as np

sys.path.insert(0, "/opt/trn_rl_repo")

import concourse.bass as bass
import concourse.bacc as bacc
import concourse.tile as tile
from concourse import mybir
from concourse.bass_utils import run_bass_kernel_spmd

import ml_dtypes

MM_DT = "f16"  # "f32" | "bf16" | "f16" — dtype of the matmul path.
# f16 measured: rel err 4.6e-4, ~410 us/iter; f32: rel err 1.5e-7, ~720 us.
# timing-only ablation switches (break correctness when nonzero)
SKIP_P1 = False      # skip phase-1 segment sum
SKIP_REV = False     # skip rev indirect gathers
SKIP_LIN = False     # skip transpose+linear (write muv directly)
SB_BUFS = 4          # sbuf working-pool depth
PS_BUFS = (2, 2, 2, 2)  # psum bufs: mv, pv, tr, out (sum of banks <= 8)
TCOPY_ACT = False    # PSUM->SBUF transpose copy on ScalarE instead of DVE
BIAS_PE = True       # bias via K=1 matmul on PE; out copy on ScalarE

N_NODES = 50000
N_EDGES = 300000
HID = 256
NC = 8
P = 128
NPC = N_NODES // NC          # 6250 nodes per core
NBLK = (NPC + P - 1) // P    # 49 blocks of 128 node lanes per core
PAD_LANE = 200.0             # sentinel lane value -> one-hot row of zeros


def _group_slots(node_ids):
    """Group edges by (core, block) of node ownership; assign (chunk, lane) slots.

    Returns (order, core, blk, j, p, lane, CPB): arrays over edges in grouped
    order; edge order[i] sits at core[i], block blk[i], chunk j[i], lane p[i],
    and selects node lane lane[i] within the block. CPB = uniform chunks/block.
    """
    c = node_ids // NPC
    loc = node_ids - c * NPC
    blk = loc >> 7
    lane = loc & 127
    g = c * NBLK + blk
    order = np.argsort(g, kind="stable")
    gs = g[order]
    starts = np.searchsorted(gs, np.arange(NC * NBLK))
    counts = np.diff(np.append(starts, node_ids.shape[0]))
    CPB = int(-(-counts.max() // P))
    rank = np.arange(node_ids.shape[0]) - starts[gs]
    j = rank >> 7
    p = rank & 127
    return order, c[order], blk[order], j, p, lane[order], int(CPB)


def prepare(E, edge_index, rev_index, W, b):
    """Host-side sharding. Returns (in_maps, meta)."""
    E = np.ascontiguousarray(E, dtype=np.float32)
    src = np.asarray(edge_index[0], dtype=np.int64)
    dest = np.asarray(edge_index[1], dtype=np.int64)
    rev = np.asarray(rev_index, dtype=np.int64)
    W = np.asarray(W, dtype=np.float32)
    b = np.asarray(b, dtype=np.float32)

    # ---- phase 1: dest-grouped permuted sharding of E ----
    o1, c1, blk1, j1, p1, lane1, CPB1 = _group_slots(dest)
    R1 = NBLK * CPB1 * P  # rows per core in E_p1
    row1 = blk1 * (CPB1 * P) + j1 * P + p1
    col1 = blk1 * CPB1 + j1

    # ---- phase 2: src-grouped slots ----
    o2, c2, blk2, j2, p2, lane2, CPB2 = _group_slots(src)
    R2 = NBLK * CPB2 * P
    row2 = blk2 * (CPB2 * P) + j2 * P + p2
    col2 = blk2 * CPB2 + j2

    npmm = {"f32": np.float32, "bf16": ml_dtypes.bfloat16,
        "f16": np.float16}[MM_DT]
    Wt_stack = np.ascontiguousarray(W.T.reshape(2, P, HID)).astype(npmm)
    bias_tile = np.ascontiguousarray(np.broadcast_to(b, (P, HID)))
    iota_row = np.ascontiguousarray(
        np.broadcast_to(np.arange(P, dtype=np.float32), (P, P))).astype(npmm)
    iota_col = np.arange(P, dtype=np.float32).reshape(P, 1).astype(npmm)
    identity = np.eye(P, dtype=np.float32).astype(npmm)

    in_maps = []
    metas = []
    for c in range(NC):
        m1 = c1 == c
        e1 = o1[m1]
        E_p1 = np.zeros((R1, HID), npmm)
        E_p1[row1[m1]] = E[e1].astype(npmm)
        dest_f32 = np.full((P, NBLK * CPB1), PAD_LANE, npmm)
        dest_f32[p1[m1], col1[m1]] = lane1[m1].astype(npmm)

        m2 = c2 == c
        e2 = o2[m2]
        rev_ids = np.zeros((P, NBLK * CPB2), np.int32)
        rev_ids[p2[m2], col2[m2]] = rev[e2].astype(np.int32)
        src_bc_row = np.full(R2, PAD_LANE, npmm)
        src_bc_row[row2[m2]] = lane2[m2].astype(npmm)
        # [128, R2] broadcast of per-slot src lane (free dim = slot e-lane)
        src_bc = np.ascontiguousarray(
            np.broadcast_to(src_bc_row, (P, R2)))

        in_maps.append({
            "E_full": E,
            "E_p1": E_p1,
            "dest_f32": dest_f32,
            "rev_ids": rev_ids,
            "src_bc": src_bc,
            "Wt": Wt_stack,
            "bias": bias_tile,
            "iota_row": iota_row,
            "iota_col": iota_col,
            "ident": identity,
        })
        metas.append({"e2": e2, "row2": row2[m2]})

    meta = {"CPB1": CPB1, "CPB2": CPB2, "metas": metas}
    return in_maps, meta


def build_program(CPB1, CPB2, reps=1):
    R1 = NBLK * CPB1 * P
    R2 = NBLK * CPB2 * P
    f32 = mybir.dt.float32
    dmm = {"f32": f32, "bf16": mybir.dt.bfloat16,
           "f16": mybir.dt.float16}[MM_DT]
    nc = bacc.Bacc("TRN2", target_bir_lowering=False, debug=False,
                   num_devices=NC)
    E_full = nc.dram_tensor("E_full", [N_EDGES, HID], f32,
                            kind="ExternalInput").ap()
    E_p1 = nc.dram_tensor("E_p1", [R1, HID], dmm, kind="ExternalInput").ap()
    dest_f32 = nc.dram_tensor("dest_f32", [P, NBLK * CPB1], dmm,
                              kind="ExternalInput").ap()
    rev_ids = nc.dram_tensor("rev_ids", [P, NBLK * CPB2], mybir.dt.int32,
                             kind="ExternalInput").ap()
    src_bc = nc.dram_tensor("src_bc", [P, R2], dmm, kind="ExternalInput").ap()
    Wt = nc.dram_tensor("Wt", [2, P, HID], dmm, kind="ExternalInput").ap()
    bias = nc.dram_tensor("bias", [P, HID], f32, kind="ExternalInput").ap()
    iota_row = nc.dram_tensor("iota_row", [P, P], dmm,
                              kind="ExternalInput").ap()
    iota_col = nc.dram_tensor("iota_col", [P, 1], dmm,
                              kind="ExternalInput").ap()
    ident = nc.dram_tensor("ident", [P, P], dmm, kind="ExternalInput").ap()
    out = nc.dram_tensor("out", [R2, HID], f32, kind="ExternalOutput").ap()

    with tile.TileContext(nc) as tc:
        with ExitStack() as ctx:
            const = ctx.enter_context(tc.tile_pool(name="const", bufs=1))
            sb = ctx.enter_context(tc.tile_pool(name="sb", bufs=SB_BUFS))
            mvp = ctx.enter_context(tc.tile_pool(name="mv", bufs=1))
            ps_mv = ctx.enter_context(
                tc.tile_pool(name="ps_mv", bufs=PS_BUFS[0], space="PSUM"))
            ps_pv = ctx.enter_context(
                tc.tile_pool(name="ps_pv", bufs=PS_BUFS[1], space="PSUM"))
            ps_tr = ctx.enter_context(
                tc.tile_pool(name="ps_tr", bufs=PS_BUFS[2], space="PSUM"))
            ps_out = ctx.enter_context(
                tc.tile_pool(name="ps_out", bufs=PS_BUFS[3], space="PSUM"))

            # constants
            wt0 = const.tile([P, HID], dmm)
            nc.sync.dma_start(out=wt0[:], in_=Wt[0])
            wt1 = const.tile([P, HID], dmm)
            nc.sync.dma_start(out=wt1[:], in_=Wt[1])
            bias_t = const.tile([P, HID], f32)
            nc.sync.dma_start(out=bias_t[:], in_=bias[:])
            iota_r = const.tile([P, P], dmm)
            nc.sync.dma_start(out=iota_r[:], in_=iota_row[:])
            iota_c = const.tile([P, 1], dmm)
            nc.sync.dma_start(out=iota_c[:], in_=iota_col[:])
            ident_t = const.tile([P, P], dmm)
            nc.sync.dma_start(out=ident_t[:], in_=ident[:])
            dest_t = const.tile([P, NBLK * CPB1], dmm)
            nc.sync.dma_start(out=dest_t[:], in_=dest_f32[:])
            ones_t = const.tile([1, P], dmm)
            nc.gpsimd.memset(ones_t[:], 1.0)
            brow_t = const.tile([1, HID], dmm)
            nc.gpsimd.dma_start(out=brow_t[:], in_=bias[0:1, :])
            rev_t = const.tile([P, NBLK * CPB2], mybir.dt.int32)
            nc.sync.dma_start(out=rev_t[:], in_=rev_ids[:])

            mv_all = mvp.tile([P, NBLK * HID], dmm)  # resident M_v

            for _rep in range(reps):
                _emit_body(nc, tc, locals(), CPB1, CPB2)
    nc.compile()
    return nc


def _emit_body(nc, tc, env, CPB1, CPB2):
    f32 = mybir.dt.float32
    dmm = {"f32": f32, "bf16": mybir.dt.bfloat16,
           "f16": mybir.dt.float16}[MM_DT]
    sb, mv_all = env["sb"], env["mv_all"]
    ps_mv, ps_pv, ps_tr, ps_out = (env["ps_mv"], env["ps_pv"], env["ps_tr"],
                                   env["ps_out"])
    E_p1, E_full, src_bc, out = (env["E_p1"], env["E_full"], env["src_bc"],
                                 env["out"])
    dest_t, rev_t = env["dest_t"], env["rev_t"]
    iota_r, iota_c, ident_t = env["iota_r"], env["iota_c"], env["ident_t"]
    wt0, wt1, bias_t = env["wt0"], env["wt1"], env["bias_t"]
    ones_t, brow_t = env["ones_t"], env["brow_t"]
    for bb in range(NBLK):
        if True:
            # ---------------- phase 1: segment sum ----------------
            if not SKIP_P1:
                h_blk = sb.tile([P, CPB1 * HID], dmm, tag="h_blk")
                base = bb * CPB1 * P
                nc.sync.dma_start(
                    out=h_blk[:].rearrange("p (j d) -> p j d", j=CPB1),
                    in_=E_p1[base:base + CPB1 * P, :].rearrange(
                        "(j p) d -> p j d", p=P))
                nc.scalar.activation(h_blk[:], h_blk[:],
                                     mybir.ActivationFunctionType.Relu)
                mv_ps = ps_mv.tile([P, HID], f32, space="PSUM")
                for j in range(CPB1):
                    s_t = sb.tile([P, P], dmm, tag="s_t")
                    col = bb * CPB1 + j
                    nc.vector.tensor_tensor(
                        out=s_t[:],
                        in0=dest_t[:, col:col + 1].to_broadcast([P, P]),
                        in1=iota_r[:],
                        op=mybir.AluOpType.is_equal)
                    nc.tensor.matmul(
                        out=mv_ps[:], lhsT=s_t[:],
                        rhs=h_blk[:, j * HID:(j + 1) * HID],
                        start=(j == 0), stop=(j == CPB1 - 1))
                nc.vector.tensor_copy(
                    out=mv_all[:, bb * HID:(bb + 1) * HID], in_=mv_ps[:])

            # ------------- phase 2: gather-subtract-linear (same block) -------------
            if True:
                er_blk = sb.tile([P, CPB2 * HID], f32, tag="er_blk")
                if not SKIP_REV:
                    for j in range(CPB2):
                        col = bb * CPB2 + j
                        nc.gpsimd.indirect_dma_start(
                            out=er_blk[:, j * HID:(j + 1) * HID],
                            out_offset=None,
                            in_=E_full[:],
                            in_offset=bass.IndirectOffsetOnAxis(
                                ap=rev_t[:, col:col + 1], axis=0))
                else:
                    nc.gpsimd.memset(er_blk[:], 0.0)
                nc.scalar.activation(er_blk[:], er_blk[:],
                                     mybir.ActivationFunctionType.Relu)
                x_blk = sb.tile([P, CPB2 * P], dmm, tag="x_blk")
                base = bb * CPB2 * P
                nc.sync.dma_start(out=x_blk[:],
                                  in_=src_bc[:, base:base + CPB2 * P])
                r_blk = sb.tile([P, CPB2 * P], dmm, tag="r_blk")
                nc.vector.tensor_tensor(
                    out=r_blk[:], in0=x_blk[:],
                    in1=iota_c[:, 0:1].to_broadcast([P, CPB2 * P]),
                    op=mybir.AluOpType.is_equal)
                out_blk = sb.tile([P, CPB2 * HID], f32, tag="out_blk")
                for j in range(CPB2):
                    pv_ps = ps_pv.tile([P, HID], f32, space="PSUM")
                    nc.tensor.matmul(
                        out=pv_ps[:], lhsT=r_blk[:, j * P:(j + 1) * P],
                        rhs=mv_all[:, bb * HID:(bb + 1) * HID],
                        start=True, stop=True)
                    muv = sb.tile([P, HID], dmm, tag="muv")
                    nc.vector.tensor_tensor(
                        out=muv[:], in0=pv_ps[:],
                        in1=er_blk[:, j * HID:(j + 1) * HID],
                        op=mybir.AluOpType.subtract)
                    if not SKIP_LIN:
                        tr_ps = ps_tr.tile([P, HID], dmm, space="PSUM")
                        nc.tensor.transpose(tr_ps[:, 0:P], muv[:, 0:P],
                                            ident_t[:])
                        nc.tensor.transpose(tr_ps[:, P:HID], muv[:, P:HID],
                                            ident_t[:])
                        t_sb = sb.tile([P, HID], dmm, tag="t_sb")
                        if TCOPY_ACT:
                            nc.scalar.activation(
                                t_sb[:], tr_ps[:],
                                mybir.ActivationFunctionType.Copy)
                        else:
                            nc.vector.tensor_copy(out=t_sb[:], in_=tr_ps[:])
                        out_ps = ps_out.tile([P, HID], f32, space="PSUM")
                        nc.tensor.matmul(out=out_ps[:], lhsT=t_sb[:, 0:P],
                                         rhs=wt0[:], start=True, stop=False)
                        nc.tensor.matmul(out=out_ps[:], lhsT=t_sb[:, P:HID],
                                         rhs=wt1[:], start=False,
                                         stop=not BIAS_PE)
                        if BIAS_PE:
                            nc.tensor.matmul(out=out_ps[:], lhsT=ones_t[:],
                                             rhs=brow_t[:], start=False,
                                             stop=True)
                            nc.scalar.activation(
                                out_blk[:, j * HID:(j + 1) * HID], out_ps[:],
                                mybir.ActivationFunctionType.Copy)
                        else:
                            nc.vector.tensor_tensor(
                                out=out_blk[:, j * HID:(j + 1) * HID],
                                in0=out_ps[:], in1=bias_t[:],
                                op=mybir.AluOpType.add)
                    else:
                        nc.vector.tensor_copy(
                            out=out_blk[:, j * HID:(j + 1) * HID], in_=muv[:])
                nc.sync.dma_start(
                    out=out[base:base + CPB2 * P, :].rearrange(
                        "(j p) d -> p j d", p=P),
                    in_=out_blk[:].rearrange("p (j d) -> p j d", j=CPB2))


def assemble(results, meta):
    out_full = np.empty((N_EDGES, HID), np.float32)
    for c in range(NC):
        mc = meta["metas"][c]
        out_full[mc["e2"]] = results[c]["out"][mc["row2"]]
    return out_full


def kernel(E, edge_index, rev_index, W, b):
    in_maps, meta = prepare(E, edge_index, rev_index, W, b)
    nc = build_program(meta["CPB1"], meta["CPB2"])
    res = run_bass_kernel_spmd(nc, in_maps, list(range(NC)))
    return assemble(res.results, meta)


# revision 22
# speedup vs baseline: 1.4359x; 1.4359x over previous
"""Trainium2 Bass kernel for a Chemprop GNN message-passing layer.

Reference computation (single layer, n_nodes=50000, n_edges=300000, hidden=256):
    H   = relu(E)                                  # [E, 256]
    M_v = segment_sum(H, dest, n_nodes)            # [V, 256]
    out = (M_v[src] - H[rev]) @ W.T + b            # [E, 256]

Distribution over 8 NeuronCores (zero collectives):
  * Nodes are sharded: core c owns node range [c*6250, (c+1)*6250), padded to
    49 blocks of 128 lanes.
  * Phase 1 (segment sum): edges are grouped by dest-node ownership on the
    host; each core receives its edges' E-rows PRE-PERMUTED into
    (block, chunk, lane) slot order (a pure permutation + zero padding of E,
    i.e. a "dest-sorted edge sharding"). The device streams them
    contiguously, applies relu, and accumulates per 128-node block with
    one-hot selection matmuls: M_v_block += S_chunk.T @ H_chunk where
    S[e, n] = (dest_lane[e] == n), built on-device via is_equal vs an iota
    row. M_v lives entirely in SBUF (49 blocks x [128, 256]).
  * Phase 2 (gather-subtract-linear): edges are grouped by src-node
    ownership, so M_v[src] expansion is a local one-hot matmul
    Pv = R.T @ Mv_block with R[n, e] = (src_lane[e] == n). Only the
    reverse-edge term needs indirect gathers: E[rev] rows are fetched
    128-rows-per-instruction from a full replica of E in each core's DRAM.
    M_uv = Pv - relu(E[rev]) is transposed on the PE (two 128x128
    transposes) and multiplied by W.T via two accumulating matmuls; bias is
    fused into the PSUM->SBUF copy. Output rows are written contiguously in
    slot order; the host scatters them back to original edge order.
"""

import sys
from contextlib import ExitStack

import numpy as np

sys.path.insert(0, "/opt/trn_rl_repo")

import concourse.bass as bass
import concourse.bacc as bacc
import concourse.tile as tile
from concourse import mybir
from concourse.bass_utils import run_bass_kernel_spmd

import ml_dtypes

MM_DT = "f16"  # "f32" | "bf16" | "f16" — dtype of the matmul path.
# f16 measured: rel err 4.6e-4, ~410 us/iter; f32: rel err 1.5e-7, ~720 us.
# timing-only ablation switches (break correctness when nonzero)
SKIP_P1 = False      # skip phase-1 segment sum
SKIP_REV = False     # skip rev indirect gathers
SKIP_LIN = False     # skip transpose+linear (write muv directly)
SB_BUFS = 4          # sbuf working-pool depth
PS_BUFS = (2, 2, 2, 2)  # psum bufs: mv, pv, tr, out (sum of banks <= 8)
TCOPY_ACT = False    # PSUM->SBUF transpose copy on ScalarE instead of DVE
BIAS_PE = False      # measured worse on HW (ACT copy slow); keep DVE bias-add

N_NODES = 50000
N_EDGES = 300000
HID = 256
NC = 8
P = 128
NPC = N_NODES // NC          # 6250 nodes per core
NBLK = (NPC + P - 1) // P    # 49 blocks of 128 node lanes per core
PAD_LANE = 200.0             # sentinel lane value -> one-hot row of zeros


def _group_slots(node_ids):
    """Group edges by (core, block) of node ownership; assign (chunk, lane) slots.

    Returns (order, core, blk, j, p, lane, CPB): arrays over edges in grouped
    order; edge order[i] sits at core[i], block blk[i], chunk j[i], lane p[i],
    and selects node lane lane[i] within the block. CPB = uniform chunks/block.
    """
    c = node_ids // NPC
    loc = node_ids - c * NPC
    blk = loc >> 7
    lane = loc & 127
    g = c * NBLK + blk
    order = np.argsort(g, kind="stable")
    gs = g[order]
    starts = np.searchsorted(gs, np.arange(NC * NBLK))
    counts = np.diff(np.append(starts, node_ids.shape[0]))
    CPB = int(-(-counts.max() // P))
    rank = np.arange(node_ids.shape[0]) - starts[gs]
    j = rank >> 7
    p = rank & 127
    return order, c[order], blk[order], j, p, lane[order], int(CPB)


def prepare(E, edge_index, rev_index, W, b):
    """Host-side sharding. Returns (in_maps, meta)."""
    E = np.ascontiguousarray(E, dtype=np.float32)
    src = np.asarray(edge_index[0], dtype=np.int64)
    dest = np.asarray(edge_index[1], dtype=np.int64)
    rev = np.asarray(rev_index, dtype=np.int64)
    W = np.asarray(W, dtype=np.float32)
    b = np.asarray(b, dtype=np.float32)

    # ---- phase 1: dest-grouped permuted sharding of E ----
    o1, c1, blk1, j1, p1, lane1, CPB1 = _group_slots(dest)
    R1 = NBLK * CPB1 * P  # rows per core in E_p1
    row1 = blk1 * (CPB1 * P) + j1 * P + p1
    col1 = blk1 * CPB1 + j1

    # ---- phase 2: src-grouped slots ----
    o2, c2, blk2, j2, p2, lane2, CPB2 = _group_slots(src)
    R2 = NBLK * CPB2 * P
    row2 = blk2 * (CPB2 * P) + j2 * P + p2
    col2 = blk2 * CPB2 + j2

    npmm = {"f32": np.float32, "bf16": ml_dtypes.bfloat16,
        "f16": np.float16}[MM_DT]
    Wt_stack = np.ascontiguousarray(W.T.reshape(2, P, HID)).astype(npmm)
    bias_tile = np.ascontiguousarray(np.broadcast_to(b, (P, HID)))
    iota_row = np.ascontiguousarray(
        np.broadcast_to(np.arange(P, dtype=np.float32), (P, P))).astype(npmm)
    iota_col = np.arange(P, dtype=np.float32).reshape(P, 1).astype(npmm)
    identity = np.eye(P, dtype=np.float32).astype(npmm)

    in_maps = []
    metas = []
    for c in range(NC):
        m1 = c1 == c
        e1 = o1[m1]
        E_p1 = np.zeros((R1, HID), npmm)
        E_p1[row1[m1]] = E[e1].astype(npmm)
        dest_f32 = np.full((P, NBLK * CPB1), PAD_LANE, npmm)
        dest_f32[p1[m1], col1[m1]] = lane1[m1].astype(npmm)

        m2 = c2 == c
        e2 = o2[m2]
        rev_ids = np.zeros((P, NBLK * CPB2), np.int32)
        rev_ids[p2[m2], col2[m2]] = rev[e2].astype(np.int32)
        src_bc_row = np.full(R2, PAD_LANE, npmm)
        src_bc_row[row2[m2]] = lane2[m2].astype(npmm)
        # [128, R2] broadcast of per-slot src lane (free dim = slot e-lane)
        src_bc = np.ascontiguousarray(
            np.broadcast_to(src_bc_row, (P, R2)))

        in_maps.append({
            "E_full": E if MM_DT == "f32" else E.astype(npmm),
            "E_p1": E_p1,
            "dest_f32": dest_f32,
            "rev_ids": rev_ids,
            "src_bc": src_bc,
            "Wt": Wt_stack,
            "bias": bias_tile,
            "iota_row": iota_row,
            "iota_col": iota_col,
            "ident": identity,
        })
        metas.append({"e2": e2, "row2": row2[m2]})

    meta = {"CPB1": CPB1, "CPB2": CPB2, "metas": metas}
    return in_maps, meta


def build_program(CPB1, CPB2, reps=1):
    R1 = NBLK * CPB1 * P
    R2 = NBLK * CPB2 * P
    f32 = mybir.dt.float32
    dmm = {"f32": f32, "bf16": mybir.dt.bfloat16,
           "f16": mybir.dt.float16}[MM_DT]
    nc = bacc.Bacc("TRN2", target_bir_lowering=False, debug=False,
                   num_devices=NC)
    E_full = nc.dram_tensor("E_full", [N_EDGES, HID], dmm,
                            kind="ExternalInput").ap()
    E_p1 = nc.dram_tensor("E_p1", [R1, HID], dmm, kind="ExternalInput").ap()
    dest_f32 = nc.dram_tensor("dest_f32", [P, NBLK * CPB1], dmm,
                              kind="ExternalInput").ap()
    rev_ids = nc.dram_tensor("rev_ids", [P, NBLK * CPB2], mybir.dt.int32,
                             kind="ExternalInput").ap()
    src_bc = nc.dram_tensor("src_bc", [P, R2], dmm, kind="ExternalInput").ap()
    Wt = nc.dram_tensor("Wt", [2, P, HID], dmm, kind="ExternalInput").ap()
    bias = nc.dram_tensor("bias", [P, HID], f32, kind="ExternalInput").ap()
    iota_row = nc.dram_tensor("iota_row", [P, P], dmm,
                              kind="ExternalInput").ap()
    iota_col = nc.dram_tensor("iota_col", [P, 1], dmm,
                              kind="ExternalInput").ap()
    ident = nc.dram_tensor("ident", [P, P], dmm, kind="ExternalInput").ap()
    out = nc.dram_tensor("out", [R2, HID], f32, kind="ExternalOutput").ap()

    with tile.TileContext(nc) as tc:
        with ExitStack() as ctx:
            const = ctx.enter_context(tc.tile_pool(name="const", bufs=1))
            sb = ctx.enter_context(tc.tile_pool(name="sb", bufs=SB_BUFS))
            mvp = ctx.enter_context(tc.tile_pool(name="mv", bufs=1))
            ps_mv = ctx.enter_context(
                tc.tile_pool(name="ps_mv", bufs=PS_BUFS[0], space="PSUM"))
            ps_pv = ctx.enter_context(
                tc.tile_pool(name="ps_pv", bufs=PS_BUFS[1], space="PSUM"))
            ps_tr = ctx.enter_context(
                tc.tile_pool(name="ps_tr", bufs=PS_BUFS[2], space="PSUM"))
            ps_out = ctx.enter_context(
                tc.tile_pool(name="ps_out", bufs=PS_BUFS[3], space="PSUM"))

            # constants
            wt0 = const.tile([P, HID], dmm)
            nc.sync.dma_start(out=wt0[:], in_=Wt[0])
            wt1 = const.tile([P, HID], dmm)
            nc.sync.dma_start(out=wt1[:], in_=Wt[1])
            bias_t = const.tile([P, HID], f32)
            nc.sync.dma_start(out=bias_t[:], in_=bias[:])
            iota_r = const.tile([P, P], dmm)
            nc.sync.dma_start(out=iota_r[:], in_=iota_row[:])
            iota_c = const.tile([P, 1], dmm)
            nc.sync.dma_start(out=iota_c[:], in_=iota_col[:])
            ident_t = const.tile([P, P], dmm)
            nc.sync.dma_start(out=ident_t[:], in_=ident[:])
            dest_t = const.tile([P, NBLK * CPB1], dmm)
            nc.sync.dma_start(out=dest_t[:], in_=dest_f32[:])
            ones_t = const.tile([1, P], dmm)
            nc.gpsimd.memset(ones_t[:], 1.0)
            brow_t = const.tile([1, HID], dmm)
            nc.gpsimd.dma_start(out=brow_t[:], in_=bias[0:1, :])
            rev_t = const.tile([P, NBLK * CPB2], mybir.dt.int32)
            nc.sync.dma_start(out=rev_t[:], in_=rev_ids[:])

            mv_all = mvp.tile([P, NBLK * HID], dmm)  # resident M_v

            for _rep in range(reps):
                _emit_body(nc, tc, locals(), CPB1, CPB2)
    nc.compile()
    return nc


def _emit_body(nc, tc, env, CPB1, CPB2):
    f32 = mybir.dt.float32
    dmm = {"f32": f32, "bf16": mybir.dt.bfloat16,
           "f16": mybir.dt.float16}[MM_DT]
    sb, mv_all = env["sb"], env["mv_all"]
    ps_mv, ps_pv, ps_tr, ps_out = (env["ps_mv"], env["ps_pv"], env["ps_tr"],
                                   env["ps_out"])
    E_p1, E_full, src_bc, out = (env["E_p1"], env["E_full"], env["src_bc"],
                                 env["out"])
    dest_t, rev_t = env["dest_t"], env["rev_t"]
    iota_r, iota_c, ident_t = env["iota_r"], env["iota_c"], env["ident_t"]
    wt0, wt1, bias_t = env["wt0"], env["wt1"], env["bias_t"]
    ones_t, brow_t = env["ones_t"], env["brow_t"]
    for bb in range(NBLK):
        if True:
            # ---------------- phase 1: segment sum ----------------
            if not SKIP_P1:
                h_blk = sb.tile([P, CPB1 * HID], dmm, tag="h_blk")
                base = bb * CPB1 * P
                nc.sync.dma_start(
                    out=h_blk[:].rearrange("p (j d) -> p j d", j=CPB1),
                    in_=E_p1[base:base + CPB1 * P, :].rearrange(
                        "(j p) d -> p j d", p=P))
                nc.scalar.activation(h_blk[:], h_blk[:],
                                     mybir.ActivationFunctionType.Relu)
                mv_ps = ps_mv.tile([P, HID], f32, space="PSUM")
                for j in range(CPB1):
                    s_t = sb.tile([P, P], dmm, tag="s_t")
                    col = bb * CPB1 + j
                    nc.vector.tensor_tensor(
                        out=s_t[:],
                        in0=dest_t[:, col:col + 1].to_broadcast([P, P]),
                        in1=iota_r[:],
                        op=mybir.AluOpType.is_equal)
                    nc.tensor.matmul(
                        out=mv_ps[:], lhsT=s_t[:],
                        rhs=h_blk[:, j * HID:(j + 1) * HID],
                        start=(j == 0), stop=(j == CPB1 - 1))
                nc.vector.tensor_copy(
                    out=mv_all[:, bb * HID:(bb + 1) * HID], in_=mv_ps[:])

            # ------------- phase 2: gather-subtract-linear (same block) -------------
            if True:
                er_blk = sb.tile([P, CPB2 * HID], dmm, tag="er_blk")
                if not SKIP_REV:
                    for j in range(CPB2):
                        col = bb * CPB2 + j
                        nc.gpsimd.indirect_dma_start(
                            out=er_blk[:, j * HID:(j + 1) * HID],
                            out_offset=None,
                            in_=E_full[:],
                            in_offset=bass.IndirectOffsetOnAxis(
                                ap=rev_t[:, col:col + 1], axis=0))
                else:
                    nc.gpsimd.memset(er_blk[:], 0.0)
                nc.scalar.activation(er_blk[:], er_blk[:],
                                     mybir.ActivationFunctionType.Relu)
                x_blk = sb.tile([P, CPB2 * P], dmm, tag="x_blk")
                base = bb * CPB2 * P
                nc.sync.dma_start(out=x_blk[:],
                                  in_=src_bc[:, base:base + CPB2 * P])
                r_blk = sb.tile([P, CPB2 * P], dmm, tag="r_blk")
                nc.vector.tensor_tensor(
                    out=r_blk[:], in0=x_blk[:],
                    in1=iota_c[:, 0:1].to_broadcast([P, CPB2 * P]),
                    op=mybir.AluOpType.is_equal)
                out_blk = sb.tile([P, CPB2 * HID], f32, tag="out_blk")
                for j in range(CPB2):
                    pv_ps = ps_pv.tile([P, HID], f32, space="PSUM")
                    nc.tensor.matmul(
                        out=pv_ps[:], lhsT=r_blk[:, j * P:(j + 1) * P],
                        rhs=mv_all[:, bb * HID:(bb + 1) * HID],
                        start=True, stop=True)
                    muv = sb.tile([P, HID], dmm, tag="muv")
                    nc.vector.tensor_tensor(
                        out=muv[:], in0=pv_ps[:],
                        in1=er_blk[:, j * HID:(j + 1) * HID],
                        op=mybir.AluOpType.subtract)
                    if not SKIP_LIN:
                        tr_ps = ps_tr.tile([P, HID], dmm, space="PSUM")
                        nc.tensor.transpose(tr_ps[:, 0:P], muv[:, 0:P],
                                            ident_t[:])
                        nc.tensor.transpose(tr_ps[:, P:HID], muv[:, P:HID],
                                            ident_t[:])
                        t_sb = sb.tile([P, HID], dmm, tag="t_sb")
                        if TCOPY_ACT:
                            nc.scalar.activation(
                                t_sb[:], tr_ps[:],
                                mybir.ActivationFunctionType.Copy)
                        else:
                            nc.vector.tensor_copy(out=t_sb[:], in_=tr_ps[:])
                        out_ps = ps_out.tile([P, HID], f32, space="PSUM")
                        nc.tensor.matmul(out=out_ps[:], lhsT=t_sb[:, 0:P],
                                         rhs=wt0[:], start=True, stop=False)
                        nc.tensor.matmul(out=out_ps[:], lhsT=t_sb[:, P:HID],
                                         rhs=wt1[:], start=False,
                                         stop=not BIAS_PE)
                        if BIAS_PE:
                            nc.tensor.matmul(out=out_ps[:], lhsT=ones_t[:],
                                             rhs=brow_t[:], start=False,
                                             stop=True)
                            nc.scalar.activation(
                                out_blk[:, j * HID:(j + 1) * HID], out_ps[:],
                                mybir.ActivationFunctionType.Copy)
                        else:
                            nc.vector.tensor_tensor(
                                out=out_blk[:, j * HID:(j + 1) * HID],
                                in0=out_ps[:], in1=bias_t[:],
                                op=mybir.AluOpType.add)
                    else:
                        nc.vector.tensor_copy(
                            out=out_blk[:, j * HID:(j + 1) * HID], in_=muv[:])
                nc.sync.dma_start(
                    out=out[base:base + CPB2 * P, :].rearrange(
                        "(j p) d -> p j d", p=P),
                    in_=out_blk[:].rearrange("p (j d) -> p j d", j=CPB2))


def assemble(results, meta):
    out_full = np.empty((N_EDGES, HID), np.float32)
    for c in range(NC):
        mc = meta["metas"][c]
        out_full[mc["e2"]] = results[c]["out"][mc["row2"]]
    return out_full


def kernel(E, edge_index, rev_index, W, b):
    in_maps, meta = prepare(E, edge_index, rev_index, W, b)
    nc = build_program(meta["CPB1"], meta["CPB2"])
    res = run_bass_kernel_spmd(nc, in_maps, list(range(NC)))
    return assemble(res.results, meta)
